# revision 57
# baseline (speedup 1.0000x reference)
"""Trainium2 Bass kernel for MinimalLBS (B=32, T=128, N=2048, J=52, Jb=21, L=16).

Strategy: data-parallel over B across 8 NeuronCores (4 samples per core).
Host does layout-only prep (transposes / bf16 casts / augmented-contraction
rows); all FLOPs (Rodrigues, blend/pose matmuls, skinning, per-vertex matvec)
run on device.

Device math per sample b (all matmul inputs bf16, PSUM accumulation f32):
  1. Rodrigues on DVE/ACT from pose [t,63] -> pose_feature pf [t,189] (bf16),
     PE-transposed to pfT [189,t].
  2. v_posed directly in vertex-major layout: for each n-chunk (128) and
     component c: vps[n,c,t] = sum_p pd_c[p,n]*pfT[p,t] (K split 128+61)
                             + sum_l sd_c[l,n]*betasT[l,t]   (K=17, row 16 is
                               v_template via ones row in betasT)
  3. Skinning: ts[n,i,j,t] = sum_k wT[k,n]*ar[k,i,j,t], K=53 where row 52 is
     (ones x trans[t,i] at j=3) so translation is folded in. Only i<3 kept.
  4. Matvec (DVE/GpSimd): sens[n,i,t] = sum_{j<3} ts[n,i,j,t]*v3[n,j,t]
                                        + ts[n,i,3,t]
  5. DMA out as [n,i,t] (contiguous rows); host reassembles to (B,T,N,3).
"""

import sys

sys.path.insert(0, "/opt/trn_rl_repo")

import math

import ml_dtypes
import numpy as np

import concourse.bacc as bacc
import concourse.bass as bass
import concourse.mybir as mybir
import concourse.tile as tile
from concourse import bass_utils, masks

F32 = mybir.dt.float32
BF16 = mybir.dt.bfloat16
FP8W = mybir.dt.float8e5   # weights side (posedirs/shapedirs ~1e-2)
FP8A = mybir.dt.float8e4   # moving side (pose features / betas ~1)
NPBF16 = ml_dtypes.bfloat16
NPFP8W = ml_dtypes.float8_e5m2
NPFP8A = ml_dtypes.float8_e4m3

B, T, N, JB, J, L = 32, 128, 2048, 21, 52, 16
NCORES = 8
NB = B // NCORES          # samples per core
PF = JB * 9               # 189 pose-feature dims
KB0, KB1 = 128, PF - 128  # K split for the pose matmul
JA = J + 1                # joints + translation row
LA = L + 1                # betas + template row
LB = L + 3                # betas + 3 ones rows (v_template e5m2 residuals)
NCH = N // 128            # n-chunks per sample
DRPM = mybir.MatmulPerfMode.DoubleRow

_CACHED = {}


def _build_nc(taps=False):
    nc = bacc.Bacc("TRN2", target_bir_lowering=False, debug=False)

    pose_d = nc.dram_tensor("pose", [T, NB, JB, 3], F32, kind="ExternalInput")
    pd8_d = nc.dram_tensor("pd8", [NB, 128, 2, 3, N], FP8W, kind="ExternalInput")
    betat_d = nc.dram_tensor("betat", [NB, LB, T], FP8A, kind="ExternalInput")
    wt_d = nc.dram_tensor("wt", [NB, JA, N], BF16, kind="ExternalInput")
    ar_d = nc.dram_tensor("ar", [NB, JA, 3, 4, T], BF16, kind="ExternalInput")
    out_d = nc.dram_tensor("out", [NB, NCH, 128, 3, T], BF16, kind="ExternalOutput")
    if taps:
        pf_t = nc.dram_tensor("pf_t", [T, NB, JB, 9], F32, kind="ExternalOutput")
        v3_t = nc.dram_tensor("v3_t", [NB, NCH, 128, 3, T], F32,
                              kind="ExternalOutput")
        ts_t = nc.dram_tensor("ts_t", [NB, NCH, 128, 3, 4, T], F32,
                              kind="ExternalOutput")
        pft_t = nc.dram_tensor("pft_t", [NB, 128, 2, T], F32,
                               kind="ExternalOutput")

    with tile.TileContext(nc) as tc:
        with (
            tc.tile_pool(name="const", bufs=1) as p_const,
            tc.tile_pool(name="rod", bufs=1) as p_rod,
            tc.tile_pool(name="pft", bufs=3) as p_pft,
            tc.tile_pool(name="big", bufs=2) as p_big,
            tc.tile_pool(name="small", bufs=3) as p_small,
            tc.tile_pool(name="mv", bufs=14) as p_mv,
            tc.tile_pool(name="psv", bufs=2, space="PSUM") as ps_v,
            tc.tile_pool(name="psts", bufs=2, space="PSUM") as ps_ts,
        ):
            ident = p_const.tile([128, 128], BF16)
            masks.make_identity(nc, ident[:])
            cst = p_const.tile([128, 2], F32)
            nc.vector.memset(cst[:, 0:1], math.pi / 2)
            nc.vector.memset(cst[:, 1:2], 1.0)

            # ---- Rodrigues for all NB samples at once: pose [t, nb, jb, 3]
            po = p_rod.tile([T, NB, JB, 3], F32)
            nc.sync.dma_start(po[:], pose_d[:])
            sq = p_rod.tile([T, NB, JB, 3], F32)
            nc.vector.tensor_tensor(sq[:], po[:], po[:], mybir.AluOpType.mult)
            a2 = p_rod.tile([T, NB, JB], F32)
            nc.vector.tensor_tensor(
                a2[:], sq[:, :, :, 0], sq[:, :, :, 1], mybir.AluOpType.add
            )
            a2b = p_rod.tile([T, NB, JB], F32)
            nc.vector.tensor_tensor(a2b[:], a2[:], sq[:, :, :, 2], mybir.AluOpType.add)
            a2c = p_rod.tile([T, NB, JB], F32)
            nc.vector.tensor_scalar_max(a2c[:], a2b[:], 1e-16)
            ang = p_rod.tile([T, NB, JB], F32)
            nc.scalar.sqrt(ang[:], a2c[:])
            inv = p_rod.tile([T, NB, JB], F32)
            nc.vector.reciprocal(inv[:], ang[:])
            s = p_rod.tile([T, NB, JB], F32)
            nc.scalar.activation(s[:], ang[:], mybir.ActivationFunctionType.Sin)
            co = p_rod.tile([T, NB, JB], F32)
            nc.scalar.activation(
                co[:], ang[:], mybir.ActivationFunctionType.Sin, bias=cst[:, 0:1]
            )
            u = p_rod.tile([T, NB, JB], F32)
            nc.scalar.activation(
                u[:], co[:], mybir.ActivationFunctionType.Identity,
                bias=cst[:, 1:2], scale=-1.0,
            )
            ax = p_rod.tile([T, NB, JB, 3], F32)
            nc.vector.tensor_tensor(
                ax[:], po[:], inv[:].unsqueeze(3).broadcast_to((T, NB, JB, 3)),
                mybir.AluOpType.mult,
            )

            pf = p_rod.tile([T, NB, JB, 9], BF16)

            def axc(i):
                return ax[:, :, :, i]

            prods = {}
            for (a, b2), nm in [
                ((0, 0), "xx"), ((1, 1), "yy"), ((2, 2), "zz"),
                ((0, 1), "xy"), ((0, 2), "xz"), ((1, 2), "yz"),
            ]:
                t_ = p_rod.tile([T, NB, JB], F32, tag=f"pr_{nm}")
                nc.gpsimd.tensor_tensor(t_[:], axc(a), axc(b2), mybir.AluOpType.mult)
                prods[nm] = t_
            qs = {}
            for i, nm in [(0, "qx"), (1, "qy"), (2, "qz")]:
                t_ = p_rod.tile([T, NB, JB], F32, tag=f"q_{nm}")
                nc.gpsimd.tensor_tensor(t_[:], s[:], axc(i), mybir.AluOpType.mult)
                qs[nm] = t_
            os_ = {}
            for nm in ["xy", "xz", "yz"]:
                t_ = p_rod.tile([T, NB, JB], F32, tag=f"o_{nm}")
                nc.gpsimd.tensor_tensor(
                    t_[:], u[:], prods[nm][:], mybir.AluOpType.mult
                )
                os_[nm] = t_
            for di, nm in [(0, "xx"), (4, "yy"), (8, "zz")]:
                d_ = p_rod.tile([T, NB, JB], F32, tag=f"d_{nm}")
                nc.vector.tensor_scalar_add(d_[:], prods[nm][:], -1.0)
                nc.vector.tensor_tensor(
                    pf[:, :, :, di], u[:], d_[:], mybir.AluOpType.mult
                )
            for e, o_nm, q_nm, op in [
                (1, "xy", "qz", mybir.AluOpType.subtract),
                (3, "xy", "qz", mybir.AluOpType.add),
                (2, "xz", "qy", mybir.AluOpType.add),
                (6, "xz", "qy", mybir.AluOpType.subtract),
                (5, "yz", "qx", mybir.AluOpType.subtract),
                (7, "yz", "qx", mybir.AluOpType.add),
            ]:
                nc.vector.tensor_tensor(
                    pf[:, :, :, e], os_[o_nm][:], qs[q_nm][:], op
                )

            # ---- per-sample pipeline
            for nb in range(NB):
                # pfT [128, 2, T] fp8e4: K-blocks of transposed pose features.
                # Tile1 rows 61:77 carry betas, 77:80 ones (pairing with the
                # v_template e5m2 residual rows of pd8), 80:128 zero.
                pft = p_pft.tile([128, 2, T], FP8A, tag="pft")
                nc.vector.memset(pft[:], 0.0)
                pf_nb = pf[:, nb].rearrange("t j e -> t (j e)")
                tp0 = ps_v.tile([128, 3, T], BF16, tag="psv")
                nc.tensor.transpose(tp0[:, 0, :], pf_nb[:, 0:128], ident[:])
                nc.scalar.copy(pft[:, 0, :], tp0[:, 0, :])
                tp1 = ps_v.tile([128, 3, T], BF16, tag="psv")
                nc.tensor.transpose(tp1[0:KB1, 0, :], pf_nb[:, 128:PF], ident[:])
                nc.scalar.copy(pft[0:KB1, 1, :], tp1[0:KB1, 0, :])
                nc.sync.dma_start(pft[KB1 : KB1 + LB, 1, :], betat_d[nb])

                # small skinning inputs first so the SP DMA queue unblocks
                # the ts matmuls before the big pd8 transfer streams in
                wt_s = p_small.tile([JA, N], BF16, tag="wt")
                nc.sync.dma_start(wt_s[:], wt_d[nb])
                ar_s = p_small.tile([JA, 3, 4, T], BF16, tag="ar")
                nc.sync.dma_start(ar_s[:], ar_d[nb])
                pd8_s = p_big.tile([128, 2, 3, N], FP8W, tag="pd8")
                nc.sync.dma_start(pd8_s[:], pd8_d[nb])

                for nch in range(NCH):
                    n0 = nch * 128
                    # skinning ts [n, i, j, t] (issued first: its evac is the
                    # long pole, so its PSUM should fill/drain earliest)
                    ts = ps_ts.tile([128, 3, 4, T], F32, tag="psts")
                    for i in range(3):
                        nc.tensor.matmul(
                            ts[:, i],
                            wt_s[:, n0 : n0 + 128],
                            ar_s[:, i].rearrange("k j t -> k (j t)"),
                            start=True,
                            stop=True,
                        )

                    # v_posed [n, c, t]: one fp8 DoubleRow matmul per c
                    # (K-tiles 2x128 summed at 0.5 cycles/row)
                    vps = ps_v.tile([128, 3, T], F32, tag="psv")
                    for c in range(3):
                        nc.tensor.matmul(
                            vps[:, c, :],
                            pd8_s[:, :, c, n0 : n0 + 128],
                            pft[:],
                            start=True,
                            stop=True,
                            perf_mode=DRPM,
                        )

                    # matvec. For half the chunks ACT (which has slack)
                    # evacuates ts to bf16 SBUF so the DVE multiply runs in
                    # 2x mode; the other half multiplies straight from PSUM
                    # at 1x. This balances DVE ~= ACT.  v3 is copied first:
                    # on odd chunks the DVE multiply needs only v3, so this
                    # lets it start before the t3 evacuation.
                    v3 = p_mv.tile([128, 3, T], BF16, tag="v3")
                    nc.scalar.copy(v3[:], vps[:])
                    if nch % 2 == 0:
                        tsb = p_mv.tile([128, 3, 4, T], BF16, tag="tsb")
                        nc.scalar.copy(tsb[:], ts[:])
                        mul_src = tsb[:, :, 0:3, :]
                        t3_src = tsb[:, :, 3, :]
                    else:
                        t3 = p_mv.tile([128, 3, T], BF16, tag="t3")
                        nc.scalar.copy(t3[:], ts[:, :, 3, :])
                        mul_src = ts[:, :, 0:3, :]
                        t3_src = t3[:]
                    pm = p_mv.tile([128, 3, 3, T], BF16, tag="pm")
                    nc.vector.tensor_tensor(
                        pm[:],
                        mul_src,
                        v3[:].unsqueeze(1).broadcast_to((128, 3, 3, T)),
                        mybir.AluOpType.mult,
                    )
                    s1 = p_mv.tile([128, 3, T], BF16, tag="s1")
                    nc.vector.tensor_tensor(
                        s1[:], pm[:, :, 0, :], pm[:, :, 1, :], mybir.AluOpType.add
                    )
                    s2 = p_mv.tile([128, 3, T], BF16, tag="s2")
                    nc.gpsimd.tensor_tensor(
                        s2[:], s1[:], pm[:, :, 2, :], mybir.AluOpType.add
                    )
                    sens = p_mv.tile([128, 3, T], BF16, tag="sens")
                    if nch % 2 == 0:
                        nc.vector.tensor_tensor(
                            sens[:], s2[:], t3_src, mybir.AluOpType.add
                        )
                    else:
                        nc.gpsimd.tensor_tensor(
                            sens[:], s2[:], t3_src, mybir.AluOpType.add
                        )
                    nc.sync.dma_start(out_d[nb, nch], sens[:])
                    if taps:
                        nc.sync.dma_start(v3_t[nb, nch], v3[:])
                        tsc = p_mv.tile([128, 3, 4, T], F32, tag="tsc")
                        nc.scalar.copy(tsc[:], ts[:])
                        nc.sync.dma_start(ts_t[nb, nch], tsc[:])
            if taps:
                pfc = p_rod.tile([T, NB, JB, 9], F32, tag="pfc")
                nc.vector.tensor_copy(pfc[:], pf[:])
                nc.sync.dma_start(pf_t[:], pfc[:])

    nc.compile()
    return nc


def _prep_core(c, pose_body, trans, betas, A, v_template, shapedirs, posedirs,
               lbs_weights):
    bs = slice(NB * c, NB * (c + 1))
    pose = np.ascontiguousarray(
        pose_body[bs].transpose(1, 0, 2).reshape(T, NB, JB, 3)
    ).astype(np.float32)

    # pd8 [nb, k, 2, c, n]: K-tile0 = posedirs rows 0:128, K-tile1 = rows
    # 128:189, then shapedirs (L rows), then 3 v_template e5m2 residual rows
    # (paired with ones rows of betat), rest zero.
    pdc = posedirs[bs].reshape(NB, PF, N, 3)           # [nb, p, n, c]
    pd_t = pdc.transpose(0, 1, 3, 2)                   # [nb, p, c, n]
    pd8 = np.zeros((NB, 128, 2, 3, N), dtype=NPFP8W)
    pd8[:, :, 0] = pd_t[:, 0:128].astype(NPFP8W)
    pd8[:, 0:KB1, 1] = pd_t[:, 128:PF].astype(NPFP8W)
    pd8[:, KB1 : KB1 + L, 1] = shapedirs[bs].transpose(0, 3, 2, 1).astype(NPFP8W)
    v0 = v_template[bs].transpose(0, 2, 1).astype(np.float32)   # [nb, c, n]
    for r in range(3):
        q = v0.astype(NPFP8W)
        pd8[:, KB1 + L + r, 1] = q
        v0 = v0 - q.astype(np.float32)

    betat = np.empty((NB, LB, T), dtype=NPFP8A)
    betat[:, 0:L, :] = betas[bs].transpose(0, 2, 1).astype(NPFP8A)
    betat[:, L : L + 3, :] = np.ones((NB, 3, T), dtype=NPFP8A)

    wt = np.empty((NB, JA, N), dtype=NPBF16)
    wt[:, 0:J, :] = lbs_weights[bs].transpose(0, 2, 1).astype(NPBF16)
    wt[:, J, :] = np.ones((NB, N), dtype=NPBF16)

    ar = np.zeros((NB, JA, 3, 4, T), dtype=NPBF16)
    ar[:, 0:J] = A[bs, :, :, 0:3, :].transpose(0, 2, 3, 4, 1).astype(NPBF16)
    ar[:, J, :, 3, :] = trans[bs].transpose(0, 2, 1).astype(NPBF16)

    return {
        "pose": pose, "pd8": pd8, "betat": betat, "wt": wt, "ar": ar,
    }


def kernel(pose_body, trans, betas, A, v_template, shapedirs, posedirs,
           lbs_weights):
    if "nc" not in _CACHED:
        _CACHED["nc"] = _build_nc()
    nc = _CACHED["nc"]

    args = (pose_body, trans, betas, A, v_template, shapedirs, posedirs,
            lbs_weights)
    args = tuple(np.asarray(a, dtype=np.float32) for a in args)
    in_maps = [_prep_core(c, *args) for c in range(NCORES)]

    res = bass_utils.run_bass_kernel_spmd(nc, in_maps, core_ids=list(range(NCORES)))

    # out [NB, NCH, 128, 3, T] per core -> (B, T, N, 3)
    full = np.stack(
        [res.results[c]["out"].astype(np.float32) for c in range(NCORES)]
    )
    full = full.reshape(B, NCH, 128, 3, T).transpose(0, 4, 1, 2, 3)
    return np.ascontiguousarray(full.reshape(B, T, N, 3).astype(np.float32))

